# revision 5
# baseline (speedup 1.0000x reference)
"""Trainium2 Bass kernel: GQA multi-head attention block (nn_MultiHeadAttention).

Full-input contract: kernel(**inputs) takes the unsharded inputs and returns the
full [B, T, D] output. Internally shards across 8 NeuronCores as
2 (batch / data axis) x 4 (head groups / model axis): each core processes one
batch element and 12 q heads (2 kv heads) including the row-shard of the output
projection; the host sums the 4 model-parallel partial outputs per batch.

Per-core compute layout ("transposed attention"):
  - host passes x^T [D, T] so projections emit Q^T/K^T [d, t] directly
    (features on partitions) -- no on-device transposes anywhere.
  - S^T tile [tk=128, tq=512] = single matmul (contraction d=128).
  - soft logit cap: 30*tanh(logits/30); softmax uses the fixed max 30
    (tanh bounds logits to [-30,30], so no row-max pass is needed).
  - causal: upper-triangular tiles are skipped structurally; the diagonal
    band gets additive -1e9 masks (built host-side from the mask input).
  - rope: the rotate-half partition swap is done with two SBUF->SBUF DMAs
    (DVE ops require same start partition); the sign lives in the sin table.
  - softmax denominator via ones-column matmul; 1/r broadcast across
    partitions with a stride-0 DMA.
All matmul operands are float32r end-to-end (full-rate fp32, ~1.6e-4 rel).
"""

import sys
from contextlib import ExitStack
from dataclasses import dataclass

for _p in (
    "/opt/trn_rl_repo",
    "/opt/pypackages",
    "/root/.axon_site/_ro/trn_rl_repo",
    "/root/.axon_site/_ro/pypackages",
):
    if _p not in sys.path:
        sys.path.insert(0, _p)

import numpy as np  # noqa: E402

import concourse.mybir as mybir  # noqa: E402
import concourse.tile as tile  # noqa: E402
from concourse import bacc, bass_utils  # noqa: E402

MULT = 0.08838834764831845  # 1/sqrt(128)
MAXA = 30.0  # tanh logit cap
NEG = -1.0e9  # additive mask (scaled by 30 in the exp pass)
ROPE_BASE = 10000.0
HD = 128  # head dim (fixed: rope halves assume 64/64)

F32 = mybir.dt.float32
AF = mybir.ActivationFunctionType


@dataclass(frozen=True)
class Cfg:
    T: int = 1024  # tokens per core
    D: int = 6144  # model dim
    HQ: int = 12  # q heads per core
    HKV: int = 2  # kv heads per core
    KB: int = 12  # k-tiles per q-proj SBUF-accumulation block
    CHUNK: int = 512  # tq chunk width (<= 512: one PSUM bank)
    mmdt: str = "f32r"  # matmul operand dtype: "f32r" | "bf16"

    @property
    def MD(self):
        return mybir.dt.float32r if self.mmdt == "f32r" else mybir.dt.bfloat16

    @property
    def np_md(self):
        if self.mmdt == "f32r":
            return np.float32
        import ml_dtypes
        return ml_dtypes.bfloat16

    @property
    def KT(self):
        return self.D // 128

    @property
    def NT(self):
        return self.T // 128

    @property
    def NCH(self):
        return self.T // self.CHUNK

    @property
    def NPAT(self):
        return self.CHUNK // 128

    @property
    def NQD(self):
        return self.HQ * HD

    @property
    def NKD(self):
        return self.HKV * HD

    @property
    def GRP(self):
        return self.HQ // self.HKV

    @property
    def nKB(self):
        return self.KT // self.KB


FULL = Cfg()


def _rope_inplace(nc, pool, x, cos_sb, sinr_sb, c0, w, md):
    """x[:, c0:c0+w] = x*cos + half_swap(x)*sinr, in place. x is [128, T] with
    the head dim on partitions; sinr has its first 64 rows negated so the
    half-swap is a plain partition move (two SBUF->SBUF DMAs)."""
    cs = slice(c0, c0 + w)
    qrot = pool.tile([128, w], md, name="qrot", tag="qrot")
    nc.sync.dma_start(qrot[0:64, :], x[64:128, cs])
    nc.sync.dma_start(qrot[64:128, :], x[0:64, cs])
    nc.vector.tensor_mul(out=qrot[:], in0=qrot[:], in1=sinr_sb[:, cs])
    nc.vector.tensor_mul(out=x[:, cs], in0=x[:, cs], in1=cos_sb[:, cs])
    nc.vector.tensor_add(out=x[:, cs], in0=x[:, cs], in1=qrot[:])


def build_program(C: Cfg = FULL):
    nc = bacc.Bacc("TRN2", target_bir_lowering=False, debug=False)
    MD = C.MD

    xqT = nc.dram_tensor("xqT", [C.D, C.T], MD, kind="ExternalInput").ap()
    xkT = nc.dram_tensor("xkT", [C.D, C.T], MD, kind="ExternalInput").ap()
    xvT = nc.dram_tensor("xvT", [C.D, C.T], MD, kind="ExternalInput").ap()
    wq_r = nc.dram_tensor("wq_r", [C.HQ, C.KT, 128, 128], MD, kind="ExternalInput").ap()
    wk_r = nc.dram_tensor("wk_r", [C.KT, 128, C.NKD], MD, kind="ExternalInput").ap()
    wv_r = nc.dram_tensor("wv_r", [C.KT, 128, C.NKD], MD, kind="ExternalInput").ap()
    wo_g = nc.dram_tensor("wo_g", [C.NQD, C.D], MD, kind="ExternalInput").ap()
    cosT = nc.dram_tensor("cosT", [128, C.T], F32, kind="ExternalInput").ap()
    sinrT = nc.dram_tensor("sinrT", [128, C.T], F32, kind="ExternalInput").ap()
    trineg = nc.dram_tensor("trineg", [C.NPAT, 128, C.CHUNK], F32, kind="ExternalInput").ap()
    bqh = nc.dram_tensor("bqh", [128, C.HQ], F32, kind="ExternalInput").ap()
    bkh = nc.dram_tensor("bkh", [128, C.HKV], F32, kind="ExternalInput").ap()
    ones_d = nc.dram_tensor("ones_d", [128, 1], MD, kind="ExternalInput").ap()
    out = nc.dram_tensor("out", [C.T, C.D], F32, kind="ExternalOutput").ap()

    with tile.TileContext(nc) as tc:
        with ExitStack() as ctx:
            const = ctx.enter_context(tc.tile_pool(name="const", bufs=1))
            resid = ctx.enter_context(tc.tile_pool(name="resid", bufs=1))
            rope_pool = ctx.enter_context(tc.tile_pool(name="rope", bufs=2))

            cos_sb = const.tile([128, C.T], F32, name="cos", tag="cos")
            nc.sync.dma_start(cos_sb[:], cosT)
            sinr_sb = const.tile([128, C.T], F32, name="sinr", tag="sinr")
            nc.sync.dma_start(sinr_sb[:], sinrT)
            tri_sb = const.tile([128, C.NPAT, C.CHUNK], F32, name="tri", tag="tri")
            nc.sync.dma_start(tri_sb[:], trineg.transpose([1, 0, 2]))
            bq_sb = const.tile([128, C.HQ], F32, name="bq", tag="bq")
            nc.sync.dma_start(bq_sb[:], bqh)
            bk_sb = const.tile([128, C.HKV], F32, name="bk", tag="bk")
            nc.sync.dma_start(bk_sb[:], bkh)
            ones_col = const.tile([128, 1], MD, name="ones_col", tag="ones_col")
            nc.sync.dma_start(ones_col[:], ones_d)
            zero_b = const.tile([128, 1], F32, name="zero_b", tag="zero_b")
            nc.vector.memset(zero_b[:], 0.0)
            negmax_b = const.tile([128, 1], F32, name="negmax_b", tag="negmax_b")
            nc.vector.memset(negmax_b[:], -MAXA)

            kt_sb = [resid.tile([128, C.T], MD, name=f"kt{i}", tag=f"kt{i}") for i in range(C.HKV)]
            v_sb = [resid.tile([128, C.NKD], MD, name=f"v{i}", tag=f"v{i}") for i in range(C.NT)]
            qt_sb = [resid.tile([128, C.T], MD, name=f"qt{h}", tag=f"qt{h}") for h in range(C.HQ)]
            ot_sb = [resid.tile([128, C.T], MD, name=f"ot{h}", tag=f"ot{h}") for h in range(C.HQ)]

            # ---------------- K projection: KT^T[kv] = (x_k @ wk)^T, + bias, rope
            with tc.tile_pool(name="kps", bufs=1, space="PSUM") as kps, \
                 tc.tile_pool(name="kstream", bufs=3) as ks:
                kp = [
                    [kps.tile([128, C.CHUNK], F32, name=f"kp{kv}_{c}", tag=f"kp{kv}_{c}")
                     for c in range(C.NCH)]
                    for kv in range(C.HKV)
                ]
                for k in range(C.KT):
                    xk_t = ks.tile([128, C.T], MD, name="xk", tag="xk")
                    nc.sync.dma_start(xk_t[:], xkT[k * 128:(k + 1) * 128, :])
                    wk_t = ks.tile([128, C.NKD], MD, name="wk", tag="wk")
                    nc.sync.dma_start(wk_t[:], wk_r[k])
                    for kv in range(C.HKV):
                        for c in range(C.NCH):
                            nc.tensor.matmul(
                                kp[kv][c][:],
                                wk_t[:, kv * 128:(kv + 1) * 128],
                                xk_t[:, c * C.CHUNK:(c + 1) * C.CHUNK],
                                start=(k == 0),
                                stop=(k == C.KT - 1),
                            )
                for kv in range(C.HKV):
                    for c in range(C.NCH):
                        cs = slice(c * C.CHUNK, (c + 1) * C.CHUNK)
                        nc.scalar.activation(
                            kt_sb[kv][:, cs], kp[kv][c][:], AF.Identity,
                            bias=bk_sb[:, kv:kv + 1], scale=1.0,
                        )
                        _rope_inplace(nc, rope_pool, kt_sb[kv], cos_sb, sinr_sb,
                                      c * C.CHUNK, C.CHUNK, MD)

            # ---------------- V projection: V[t-tile] = x_v @ wv (natural layout)
            with tc.tile_pool(name="vps", bufs=1, space="PSUM") as vps, \
                 tc.tile_pool(name="vstream", bufs=3) as vs:
                vp = [vps.tile([128, C.NKD], F32, name=f"vp{ti}", tag=f"vp{ti}")
                      for ti in range(C.NT)]
                for k in range(C.KT):
                    xv_t = vs.tile([128, C.T], MD, name="xv", tag="xv")
                    nc.sync.dma_start(xv_t[:], xvT[k * 128:(k + 1) * 128, :])
                    wv_t = vs.tile([128, C.NKD], MD, name="wv", tag="wv")
                    nc.sync.dma_start(wv_t[:], wv_r[k])
                    for ti in range(C.NT):
                        nc.tensor.matmul(
                            vp[ti][:],
                            xv_t[:, ti * 128:(ti + 1) * 128],
                            wv_t[:],
                            start=(k == 0),
                            stop=(k == C.KT - 1),
                        )
                for ti in range(C.NT):
                    nc.scalar.activation(v_sb[ti][:], vp[ti][:], AF.Copy)

            # ---------------- Q projection: QT[h] = (x_q @ wq)^T + bias, rope
            # SBUF accumulation over k-blocks so xqT and wq stream exactly once.
            with tc.tile_pool(name="qps", bufs=4, space="PSUM") as qps, \
                 tc.tile_pool(name="xqstream", bufs=1) as xqs, \
                 tc.tile_pool(name="wqstream", bufs=3) as wqs:
                for kb in range(C.nKB):
                    xq_tiles = []
                    for i in range(C.KB):
                        t = xqs.tile([128, C.T], MD, name=f"xq{i}", tag=f"xq{i}")
                        nc.sync.dma_start(
                            t[:], xqT[(kb * C.KB + i) * 128:(kb * C.KB + i + 1) * 128, :])
                        xq_tiles.append(t)
                    for h in range(C.HQ):
                        wq_t = wqs.tile([128, C.KB, 128], MD, name="wq", tag="wq")
                        nc.sync.dma_start(
                            wq_t[:],
                            wq_r[h, kb * C.KB:(kb + 1) * C.KB].transpose([1, 0, 2]),
                        )
                        for c in range(C.NCH):
                            cs = slice(c * C.CHUNK, (c + 1) * C.CHUNK)
                            qp = qps.tile([128, C.CHUNK], F32, name="qp", tag="qp")
                            for ki in range(C.KB):
                                nc.tensor.matmul(
                                    qp[:],
                                    wq_t[:, ki, :],
                                    xq_tiles[ki][:, cs],
                                    start=(ki == 0),
                                    stop=(ki == C.KB - 1),
                                )
                            if kb == 0:
                                nc.scalar.activation(
                                    qt_sb[h][:, cs], qp[:], AF.Identity,
                                    bias=bq_sb[:, h:h + 1], scale=1.0,
                                )
                            else:
                                nc.vector.tensor_add(
                                    out=qt_sb[h][:, cs], in0=qt_sb[h][:, cs], in1=qp[:])
                            if kb == C.nKB - 1:
                                _rope_inplace(nc, rope_pool, qt_sb[h], cos_sb, sinr_sb,
                                              c * C.CHUNK, C.CHUNK, MD)

            # ---------------- Attention (per q head, transposed layout)
            with tc.tile_pool(name="aps", bufs=2, space="PSUM") as aps, \
                 tc.tile_pool(name="attn_sb", bufs=3) as asb, \
                 tc.tile_pool(name="e_sb", bufs=4) as esb:
                for h in range(C.HQ):
                    kv = h // C.GRP
                    for c in range(C.NCH):
                        cs = slice(c * C.CHUNK, (c + 1) * C.CHUNK)
                        ntk = (c + 1) * C.NPAT
                        rsum_p = aps.tile([1, C.CHUNK], F32, name="rsum", tag="rsum")
                        ot_p = aps.tile([128, C.CHUNK], F32, name="otp", tag="otp")
                        for m in range(ntk):
                            sp = aps.tile([128, C.CHUNK], F32, name="sp", tag="sp")
                            nc.tensor.matmul(
                                sp[:],
                                kt_sb[kv][:, m * 128:(m + 1) * 128],
                                qt_sb[h][:, cs],
                                start=True, stop=True,
                            )
                            tca = asb.tile([128, C.CHUNK], F32, name="tc", tag="tc")
                            nc.scalar.activation(tca[:], sp[:], AF.Tanh,
                                                 bias=zero_b[:], scale=MULT / MAXA)
                            a = m - c * C.NPAT
                            if a >= 0:
                                nc.vector.tensor_add(
                                    out=tca[:], in0=tca[:], in1=tri_sb[:, a, :])
                            e = esb.tile([128, C.CHUNK], MD, name="e", tag="e")
                            nc.scalar.activation(e[:], tca[:], AF.Exp,
                                                 scale=MAXA, bias=negmax_b[:])
                            nc.tensor.matmul(
                                rsum_p[:], ones_col[:], e[:],
                                start=(m == 0), stop=(m == ntk - 1),
                            )
                            nc.tensor.matmul(
                                ot_p[:],
                                v_sb[m][:, kv * 128:(kv + 1) * 128],
                                e[:],
                                start=(m == 0), stop=(m == ntk - 1),
                            )
                        recip = asb.tile([1, C.CHUNK], F32, name="recip", tag="recip")
                        nc.vector.reciprocal(recip[:], rsum_p[:])
                        bc_sb = asb.tile([128, C.CHUNK], F32, name="bc_sb", tag="bc_sb")
                        nc.gpsimd.partition_broadcast(bc_sb[:], recip[:])
                        nc.vector.tensor_mul(
                            out=ot_sb[h][:, cs], in0=ot_p[:], in1=bc_sb[:])

            # ---------------- Output projection: out = O @ wo (row-sharded partial)
            with tc.tile_pool(name="ops", bufs=4, space="PSUM") as ops, \
                 tc.tile_pool(name="wostream", bufs=2) as wos, \
                 tc.tile_pool(name="obuf", bufs=3) as obp:
                for ncn in range(C.D // 512):
                    ns = slice(ncn * 512, (ncn + 1) * 512)
                    wo_tiles = []
                    for k in range(C.HQ):
                        t = wos.tile([128, 512], MD, name=f"wo{k}", tag=f"wo{k}")
                        nc.sync.dma_start(t[:], wo_g[k * 128:(k + 1) * 128, ns])
                        wo_tiles.append(t)
                    for ti in range(C.NT):
                        op = ops.tile([128, 512], F32, name="op", tag="op")
                        for k in range(C.HQ):
                            nc.tensor.matmul(
                                op[:],
                                ot_sb[k][:, ti * 128:(ti + 1) * 128],
                                wo_tiles[k][:],
                                start=(k == 0),
                                stop=(k == C.HQ - 1),
                            )
                        ob = obp.tile([128, 512], F32, name="ob", tag="ob")
                        nc.scalar.activation(ob[:], op[:], AF.Copy)
                        nc.sync.dma_start(out[ti * 128:(ti + 1) * 128, ns], ob[:])

    nc.compile()
    return nc


# ---------------------------------------------------------------------------
# Host side: sharding, rope tables, masks, gather.
# ---------------------------------------------------------------------------

def make_rope_tables(C: Cfg):
    exponents = np.arange(0, HD, 2, dtype=np.float32)
    inv_freq = (1.0 / (np.float32(ROPE_BASE) ** (exponents / np.float32(HD)))).astype(np.float32)
    t = np.arange(C.T, dtype=np.float32)
    phase = np.outer(t, inv_freq).astype(np.float32)  # [T, 64]
    phase = np.concatenate([phase, phase], axis=1)  # [T, 128]
    cosT = np.ascontiguousarray(np.cos(phase).astype(np.float32).T)  # [128, T]
    sinT = np.sin(phase).astype(np.float32).T  # [128, T]
    sinrT = sinT.copy()
    sinrT[0:64, :] *= -1.0  # sign of rotate-half folded into the table
    return cosT, np.ascontiguousarray(sinrT)


def make_trineg(C: Cfg, mask: np.ndarray):
    """Additive band masks for the diagonal tiles, from the actual mask input.
    trineg[a, p, f] = 0 if mask[f, 128*a + p] else NEG (using the first
    CHUNK-row slice; valid for any causal/tril mask)."""
    m2 = np.asarray(mask).reshape(mask.shape[-2], mask.shape[-1])
    sub = m2[:C.CHUNK, :C.NPAT * 128]  # [CHUNK(tq), NPAT*128(tk)]
    patt = sub.T.reshape(C.NPAT, 128, C.CHUNK)
    return np.where(patt, np.float32(0.0), np.float32(NEG)).astype(np.float32)


def build_in_maps(C: Cfg, query, key, value, mask, wq, bq, wk, bk, wv, bv, wo,
                  n_model: int):
    md = C.np_md
    query = np.asarray(query, dtype=np.float32)
    key = np.asarray(key, dtype=np.float32)
    value = np.asarray(value, dtype=np.float32)
    wq = np.asarray(wq, dtype=np.float32)
    wk = np.asarray(wk, dtype=np.float32)
    wv = np.asarray(wv, dtype=np.float32)
    wo = np.asarray(wo, dtype=np.float32)
    bq = np.asarray(bq, dtype=np.float32)
    bk = np.asarray(bk, dtype=np.float32)

    B = query.shape[0]
    cosT, sinrT = make_rope_tables(C)
    trineg = make_trineg(C, mask)

    xT = {}
    for b in range(B):
        xT[b] = (
            np.ascontiguousarray(query[b].T).astype(md),
            np.ascontiguousarray(key[b].T).astype(md),
            np.ascontiguousarray(value[b].T).astype(md),
        )
    gslices = {}
    for g in range(n_model):
        wq_g = wq[:, g * C.NQD:(g + 1) * C.NQD]
        wq_r = np.ascontiguousarray(
            wq_g.reshape(C.KT, 128, C.HQ, 128).transpose(2, 0, 1, 3)).astype(md)
        wk_r = np.ascontiguousarray(
            wk[:, g * C.NKD:(g + 1) * C.NKD].reshape(C.KT, 128, C.NKD)).astype(md)
        wv_r = np.ascontiguousarray(
            wv[:, g * C.NKD:(g + 1) * C.NKD].reshape(C.KT, 128, C.NKD)).astype(md)
        wo_gs = np.ascontiguousarray(wo[g * C.NQD:(g + 1) * C.NQD, :]).astype(md)
        bqh = np.ascontiguousarray(bq[g * C.NQD:(g + 1) * C.NQD].reshape(C.HQ, 128).T)
        bkh = np.ascontiguousarray(bk[g * C.NKD:(g + 1) * C.NKD].reshape(C.HKV, 128).T)
        gslices[g] = (wq_r, wk_r, wv_r, wo_gs, bqh, bkh)

    in_maps = []
    for core in range(B * n_model):
        b, g = divmod(core, n_model)
        wq_r, wk_r, wv_r, wo_gs, bqh, bkh = gslices[g]
        in_maps.append({
            "xqT": xT[b][0], "xkT": xT[b][1], "xvT": xT[b][2],
            "wq_r": wq_r, "wk_r": wk_r, "wv_r": wv_r, "wo_g": wo_gs,
            "cosT": cosT, "sinrT": sinrT, "trineg": trineg,
            "bqh": bqh, "bkh": bkh,
            "ones_d": np.ones((128, 1), dtype=md),
        })
    return in_maps


def assemble_output(C: Cfg, results, B, n_model, bv, wo):
    D = C.D
    out = np.zeros((B, C.T, D), dtype=np.float32)
    for core in range(B * n_model):
        b, g = divmod(core, n_model)
        out[b] += results[core]["out"]
    # bias_v enters linearly: rows of normalized attn weights sum to 1, so
    # O = P@V + 1*bv_exp^T exactly; fold the rank-1 term through wo on host.
    bv = np.asarray(bv, dtype=np.float32)
    wo = np.asarray(wo, dtype=np.float32)
    if np.any(bv):
        corr = np.zeros((D,), dtype=np.float32)
        for g in range(n_model):
            bv_g = bv[g * C.NKD:(g + 1) * C.NKD]
            bvexp = np.empty((C.NQD,), dtype=np.float32)
            for h in range(C.HQ):
                kvl = h // C.GRP
                bvexp[h * 128:(h + 1) * 128] = bv_g[kvl * 128:(kvl + 1) * 128]
            corr += bvexp @ wo[g * C.NQD:(g + 1) * C.NQD, :]
        out += corr[None, None, :]
    return out


_PROG_CACHE = {}


def get_program(C: Cfg = FULL):
    key = C
    if key not in _PROG_CACHE:
        _PROG_CACHE[key] = build_program(C)
    return _PROG_CACHE[key]


def kernel(query, key, value, mask, wq, bq, wk, bk, wv, bv, wo):
    C = FULL
    B = query.shape[0]
    n_model = (wq.shape[1] // HD) // C.HQ
    n_cores = B * n_model
    nc = get_program(C)
    in_maps = build_in_maps(C, query, key, value, mask, wq, bq, wk, bk, wv, bv, wo,
                            n_model)
    res = bass_utils.run_bass_kernel_spmd(nc, in_maps, core_ids=list(range(n_cores)))
    return assemble_output(C, res.results, B, n_model, bv, wo)
